# revision 28
# baseline (speedup 1.0000x reference)
"""Trainium2 Bass kernel for nn_AttentionLayer (dense_mlp, 8-core data parallel).

Reference computation (per batch b of 2048, S=200 steps, E=128):
    feat[b,s] = concat(x, t, x*t, x-t)            # [4E] with x=behaviors[b,s], t=target[b]
    h = relu(feat @ W1 + b1)                      # [64]
    w = sigmoid(h @ W2 + b2)                      # scalar
    out[b]   = sum_s w[b,s] * x[b,s]              # [128]

Algebraic folding (host side, weights only):
    feat @ W1 = x @ (W1a + W1d) + (x*t) @ W1c + t @ (W1b - W1d)
    (x*t) @ W1c = x @ (t[:,None] * W1c)
  so per batch:  h_pre = x @ Wb + c_b   with  Wb = W1ad + t_col*W1c  (per-batch weight)
                 c_b = t_b @ W1bd + b1   (per-batch bias, computed on device)

Device layout (per core, 256 batches = 51200 rows of 128):
  All compute in float32r (TF32-class PE throughput at moving dim >= 256,
  bit-identical storage to f32 so no casts anywhere):
  - 2-batch groups (400 rows): DMA 4 natural tiles [s<=128, 128],
    PE-transpose to bt [128, 400]; btx = bt * t_col (DVE, per batch).
  - h_psum[64,400] = W1ad.T @ bt + W1c.T @ btx   (shared weights, N=400)
  - per batch: ACT relu(+c_b) -> hs, w_psum[s,1] = hs_tile.T @ W2,
    ACT sigmoid -> w[s,1], then out column po[:, b] += natf_tile.T @ w
    (natural tile as the stationary operand; po is one persistent psum bank
    holding all 256 output columns in [e, b] layout).
  - epilogue: po -> sbuf, PE-transpose to [b, e], DMA out.
"""

import sys

sys.path.insert(0, "/opt/trn_rl_repo")

import numpy as np
import ml_dtypes

import concourse.bass as bass
import concourse.mybir as mybir
from concourse.tile import TileContext, add_dep_helper
from concourse.bass_utils import run_bass_kernel_spmd

F32 = mybir.dt.float32
BF16 = mybir.dt.bfloat16
AF = mybir.ActivationFunctionType

B, S, E, A = 2048, 200, 128, 64
NCORES = 8
BL = B // NCORES  # 256 batches per core
ROWS = BL * S  # 51200
G = 2  # batches per group
NG = BL // G  # 128 groups

# s-tiles within a 2-batch group (offset within 400 rows, nrows, batch idx j)
S_TILES = [(0, 128, 0), (128, 72, 0), (200, 128, 1), (328, 72, 1)]


def build_graph() -> bass.Bass:
    nc = bass.Bass()
    F32R = mybir.dt.float32r

    beh = nc.declare_dram_parameter("behaviors", [ROWS, E], F32R, isOutput=False)
    tgt = nc.declare_dram_parameter("target", [BL, E], F32R, isOutput=False)
    w1ad_d = nc.declare_dram_parameter("W1ad", [E, A], F32R, isOutput=False)
    w1c_d = nc.declare_dram_parameter("W1c", [E, A], F32R, isOutput=False)
    w1bd_d = nc.declare_dram_parameter("W1bd", [E, A], F32, isOutput=False)
    w2_d = nc.declare_dram_parameter("W2", [A, 2], F32R, isOutput=False)
    b1_d = nc.declare_dram_parameter("b1", [A, 1], F32, isOutput=False)
    b2_d = nc.declare_dram_parameter("b2c", [128, 1], F32, isOutput=False)
    eyef_d = nc.declare_dram_parameter("eyef", [128, 128], F32R, isOutput=False)
    out_d = nc.declare_dram_parameter("out", [BL, E], F32R, isOutput=True)

    with TileContext(nc) as tc:
        with (
            tc.tile_pool(name="consts", bufs=1) as cpool,
            tc.tile_pool(name="natf", bufs=3) as nfpool,
            tc.tile_pool(name="btb", bufs=2) as btbpool,
            tc.tile_pool(name="btx", bufs=2) as btxpool,
            tc.tile_pool(name="hs", bufs=2) as hspool,
            tc.tile_pool(name="ws", bufs=3) as wspool,
            tc.tile_pool(name="pbt", bufs=2, space="PSUM") as psbt,
            tc.tile_pool(name="ph", bufs=2, space="PSUM") as psh,
            tc.tile_pool(name="pw", bufs=1, space="PSUM") as psw,
            tc.tile_pool(name="po", bufs=1, space="PSUM") as pso,
        ):
            # ---- constants in ----
            w1ad = cpool.tile([E, A], F32R)
            w1c = cpool.tile([E, A], F32R)
            w1bd = cpool.tile([E, A], F32)
            w2 = cpool.tile([A, 2], F32R)
            b1 = cpool.tile([A, 1], F32)
            b2c = cpool.tile([128, 1], F32)
            eyef = cpool.tile([128, 128], F32R)
            nc.sync.dma_start(out=w1ad[:], in_=w1ad_d[:])
            nc.sync.dma_start(out=w1c[:], in_=w1c_d[:])
            nc.sync.dma_start(out=w1bd[:], in_=w1bd_d[:])
            nc.sync.dma_start(out=w2[:], in_=w2_d[:])
            nc.sync.dma_start(out=b1[:], in_=b1_d[:])
            nc.sync.dma_start(out=b2c[:], in_=b2_d[:])
            nc.sync.dma_start(out=eyef[:], in_=eyef_d[:])

            tsb0 = cpool.tile([128, E], F32R)
            tsb1 = cpool.tile([128, E], F32R)
            nc.sync.dma_start(out=tsb0[:], in_=tgt[0:128, :])
            nc.sync.dma_start(out=tsb1[:], in_=tgt[128:256, :])

            # persistent output accumulator, [e, b] layout, one psum bank
            po = pso.tile([128, 2 * BL], mybir.dt.float32, tag="po")

            # PE observers: walrus allows one sync-wait per engine-queue
            # instruction; each observer absorbs one const DMA queue's wait.
            pscr = psw.tile([128, 4], mybir.dt.float32, tag="pw")
            for k, cst in enumerate((eyef, tsb0, tsb1, w1bd, w2, w1ad, w1c)):
                p = cst.shape[0]
                nc.tensor.matmul(
                    pscr[0:1, 2 * (k % 2) : 2 * (k % 2) + 2],
                    cst[0:p, 0:1],
                    cst[0:p, 0:2],
                    start=True,
                    stop=True,
                )
            # DVE observer (b1 feeds the csb tensor_scalar below)
            scr = cpool.tile([1, 4], F32)
            nc.vector.tensor_copy(scr[0:1, 0:1], b1[0:1, 0:1])

            # ---- prologue: tT = target.T, csb = W1bd.T @ tT + b1 ----
            ptT = psbt.tile([128, G * S], F32R, tag="pbt")
            nc.tensor.transpose(ptT[:, 0:128], tsb0[:], eyef[:])
            nc.tensor.transpose(ptT[:, 128:256], tsb1[:], eyef[:])
            tTf = cpool.tile([E, BL], F32)
            nc.scalar.copy(out=tTf[:], in_=ptT[:, 0:BL])
            pC = psh.tile([A, G * S], mybir.dt.float32, tag="ph")
            nc.tensor.matmul(pC[:, 0:BL], w1bd[:], tTf[:], start=True, stop=True)
            csb = cpool.tile([A, BL], F32)
            nc.vector.tensor_scalar_add(csb[:], pC[:, 0:BL], b1[:, 0:1])
            # ACT observers (b2c / csb biases; tTf produced by ACT itself)
            scra = cpool.tile([1, 4], F32)
            nc.scalar.copy(out=scra[0:1, 0:1], in_=b2c[0:1, 0:1])
            nc.scalar.copy(out=scra[0:1, 1:2], in_=csb[0:1, 0:1])

            # ---- main loop over 2-batch groups ----
            mm1_last = {}
            for g in range(NG):
                r0 = g * G * S
                natf = []
                for k, (off, n, _) in enumerate(S_TILES):
                    t_f = nfpool.tile([n, E], F32R, tag=f"natf{k}")
                    nc.gpsimd.dma_start(out=t_f[:], in_=beh[r0 + off : r0 + off + n, :])
                    natf.append(t_f)

                pbt = psbt.tile([128, G * S], F32R, tag="pbt")
                for k, (off, n, _) in enumerate(S_TILES):
                    tr = nc.tensor.transpose(
                        pbt[:, off : off + n], natf[k][:], eyef[:n, :n]
                    )
                    if g - 1 in mm1_last:
                        # keep PE from racing ahead; collapses slot-WAR waits
                        add_dep_helper(tr.ins, mm1_last[g - 1].ins, reason="pe-pacing")
                btb = btbpool.tile([128, G * S], F32R)
                nc.vector.tensor_copy(btb[:], pbt[:])
                btx = btxpool.tile([128, G * S], F32R)
                for j in range(G):
                    bidx = g * G + j
                    nc.vector.tensor_scalar_mul(
                        btx[:, j * S : (j + 1) * S],
                        btb[:, j * S : (j + 1) * S],
                        tTf[:, bidx : bidx + 1],
                    )

                ph = psh.tile([A, G * S], mybir.dt.float32, tag="ph")
                nc.tensor.matmul(ph[:], w1ad[:], btb[:], start=True, stop=False)
                mm1_last[g] = nc.tensor.matmul(
                    ph[:], w1c[:], btx[:], start=False, stop=True
                )

                for j in range(G):
                    bidx = g * G + j
                    hs = hspool.tile([A, S], F32R, tag=f"hs{j}")
                    nc.scalar.activation(
                        hs[:],
                        ph[:, j * S : (j + 1) * S],
                        AF.Relu,
                        bias=csb[:, bidx : bidx + 1],
                        scale=1.0,
                    )
                    pw = psw.tile([128, 4], mybir.dt.float32, tag="pw")
                    nc.tensor.matmul(
                        pw[0:128, 0:2], hs[:, 0:128], w2[:], start=True, stop=True
                    )
                    nc.tensor.matmul(
                        pw[0:72, 2:4], hs[:, 128:200], w2[:], start=True, stop=True
                    )
                    ws = wspool.tile([128, 4], F32R, tag="ws")
                    nc.scalar.activation(
                        ws[:], pw[0:128, 0:4], AF.Sigmoid, bias=b2c[:, 0:1], scale=1.0
                    )
                    # f32r needs moving dim >= 2: accumulate a garbage column
                    # at po[:, 2b+1] (ws col 1/3) and drop it in the epilogue.
                    nc.tensor.matmul(
                        po[:, 2 * bidx : 2 * bidx + 2],
                        natf[2 * j][:],
                        ws[0:128, 0:2],
                        start=True,
                        stop=False,
                    )
                    nc.tensor.matmul(
                        po[:, 2 * bidx : 2 * bidx + 2],
                        natf[2 * j + 1][:],
                        ws[0:72, 2:4],
                        start=False,
                        stop=True,
                    )

            # ---- epilogue: po [e, b] -> out [b, e] ----
            obuf = cpool.tile([128, BL], F32R)
            nc.vector.tensor_copy(obuf[:], po[:, 0 : 2 * BL : 2])
            pot = psbt.tile([128, G * S], F32R, tag="pbt")
            nc.tensor.transpose(pot[:, 0:128], obuf[:, 0:128], eyef[:])
            nc.tensor.transpose(pot[:, 128:256], obuf[:, 128:256], eyef[:])
            osb = cpool.tile([128, BL], F32R)
            nc.scalar.copy(out=osb[:], in_=pot[:, 0:BL])
            nc.sync.dma_start(out=out_d[0:128, :], in_=osb[:, 0:128])
            nc.sync.dma_start(out=out_d[128:256, :], in_=osb[:, 128:256])
    _hoist_excess_waits(nc)
    return nc


# Instructions on engine queues accept only ONE sync-wait command in this
# toolchain (walrus setupSyncWait). Tile's sem assigner sometimes attaches
# more. Hoist the excess onto same-engine NoOps inserted immediately before
# the instruction — identical semantics, the wait just moves one queue slot
# earlier. DMA/Drain/branch instructions are exempt (different lowering).
_WAIT_CAP_EXEMPT = {"InstNoOp"}


def _hoist_excess_waits(nc) -> int:
    k = 0
    for fn in nc.m.functions:
        for bb in fn.blocks:
            il = bb.instructions
            out = []
            changed = False
            for inst in il:
                si = inst.sync_info
                tn = type(inst).__name__
                if si is not None and len(si.on_wait) > 1 and tn not in _WAIT_CAP_EXEMPT:
                    waits = list(si.on_wait)
                    for w in waits[:-1]:
                        nop = mybir.InstNoOp(name=f"W-hoist-{k}")
                        k += 1
                        nop.engine = inst.engine
                        nop.sync_info = mybir.SyncInfo(on_wait=[w], on_update=[])
                        out.append(nop)
                    inst.sync_info = mybir.SyncInfo(
                        on_wait=[waits[-1]], on_update=list(si.on_update)
                    )
                    changed = True
                out.append(inst)
            if changed:
                bb.instructions = out
    return k


_GRAPH_CACHE: dict = {}

# test-harness hooks (harness calls kernel() with defaults; test.py flips TRACE)
TRACE = False
TRACE_TMPDIR = None
LAST_RESULT = None


def kernel(**inputs) -> np.ndarray:
    behaviors = np.ascontiguousarray(np.asarray(inputs["behaviors"], dtype=np.float32))
    target = np.ascontiguousarray(np.asarray(inputs["target"], dtype=np.float32))
    W1 = np.asarray(inputs["W1"], dtype=np.float32)
    b1 = np.asarray(inputs["b1"], dtype=np.float32)
    W2 = np.asarray(inputs["W2"], dtype=np.float32)
    b2 = np.asarray(inputs["b2"], dtype=np.float32)

    W1a, W1b, W1c, W1d = W1[0:E], W1[E : 2 * E], W1[2 * E : 3 * E], W1[3 * E :]
    b2f = float(np.asarray(b2).reshape(-1)[0])

    if "nc" not in _GRAPH_CACHE:
        _GRAPH_CACHE["nc"] = build_graph()
    nc = _GRAPH_CACHE["nc"]

    beh_sh = behaviors.reshape(NCORES, ROWS, E)
    tgt_sh = target.reshape(NCORES, BL, E)
    in_maps = [
        dict(
            behaviors=beh_sh[i],
            target=tgt_sh[i],
            W1ad=np.ascontiguousarray(W1a + W1d),
            W1c=np.ascontiguousarray(W1c),
            W1bd=np.ascontiguousarray(W1b - W1d),
            W2=np.ascontiguousarray(np.concatenate([W2.reshape(A, 1), np.zeros((A, 1), np.float32)], axis=1)),
            b1=np.ascontiguousarray(b1.reshape(A, 1)),
            b2c=np.full((128, 1), b2f, dtype=np.float32),
            eyef=np.eye(128, dtype=np.float32),
        )
        for i in range(NCORES)
    ]
    global LAST_RESULT
    kw = {}
    if TRACE:
        kw = dict(trace=True, tmpdir=TRACE_TMPDIR)
    res = run_bass_kernel_spmd(nc, in_maps, core_ids=list(range(NCORES)), **kw)
    LAST_RESULT = res
    out = np.stack([res.results[i]["out"] for i in range(NCORES)], axis=0)
    return out.reshape(B, E).astype(np.float32)


if __name__ == "__main__":
    rng = np.random.default_rng(0)
    ins = dict(
        behaviors=rng.standard_normal((B, S, E), dtype=np.float32),
        target=rng.standard_normal((B, E), dtype=np.float32),
        W1=rng.standard_normal((4 * E, A), dtype=np.float32) * 0.04,
        b1=rng.standard_normal((A,), dtype=np.float32) * 0.04,
        W2=rng.standard_normal((A, 1), dtype=np.float32) * 0.1,
        b2=rng.standard_normal((1,), dtype=np.float32) * 0.1,
    )
    o = kernel(**ins)
    print("kernel out", o.shape, o.dtype, np.abs(o).mean())
